# revision 19
# baseline (speedup 1.0000x reference)
"""Trainium2 Bass kernel for nn_CapsuleNetwork (capsule-guided aspect routing).

Data-parallel over batch: B=128 examples sharded 16-per-core across 8 cores.
Self-contained: hardcodes all shapes; host side only shards/marshals inputs.

Math (per example, equal to reference.py up to fp associativity):
  mask = sentence != 0
  caml = embed[sentence] * alpha                      [S, D]
  primary = squash(caml @ W_s)                        [S, C]   squash over C
  asp_cap = squash(embed[aspect] @ W_a)               [C]
  v = W_att @ asp_cap                                 [C]
  scores = primary @ v ; sexp = exp(scores)*mask
  M = guide_weight @ squash(guide_capsule).T          [C, K]
  gm = softmax_k(primary @ M) * sexp[:, None]
  cc_raw[k, :] = sum_s gm[s, k] * primary[s, :]       [K, C]
  out[k] = g(scale^2 * (1/sum(sexp))^2 * |cc_raw[k]|^2)
  with g(q) = q/(1+q) * sqrt(q)/sqrt(q+eps)
(b_s / b_a are asserted zero — the task generator hardcodes them to zero.)

Implementation notes:
 * embed table is padded to 384 cols, cast to bf16 and gathered with
   dma_gather(transpose=True): the gather output IS the transposed
   [d-chunk, token] layout the PE needs — no on-chip transposes at all.
 * alpha and the squash factor are folded into per-token scalars applied
   AFTER the matmul (squash(alpha*x @ W) = (alpha-aware factor) * (x @ W)).
 * one [128,304] matmul group per token tile computes primary-linear plus
   the scores/gm projections (extra 4 columns = W_s @ [v_e | M]).
 * phase 1 (per tile): 3 matmuls, ACT square+accum, raw psum->sbuf copy.
   phase 2 (batched at [128,64]): squash factors, exp, softmax, capsule
   matmuls.  This keeps ACT on one table per phase and slashes the count
   of small vector ops.
"""

import numpy as np

B, S, D, C, K, V = 128, 512, 300, 300, 3, 30000
NCORES = 8
BPC = B // NCORES  # examples per core = 16
TPC = BPC * S  # tokens per core = 8192
NT = TPC // 128  # 64 token tiles per core
EPAD = 384  # bf16 row: 384*2 = 768B (256B-aligned)
GCH = 512  # tokens per gather call (transpose mode crashes above ~768)
NG = TPC // GCH  # 8 gather calls
SQUASH_EPS = 1e-8

_CACHE = {}


def _build():
    import concourse.bacc as bacc
    import concourse.mybir as mybir
    import concourse.tile as tile

    f32 = mybir.dt.float32
    bf16 = mybir.dt.bfloat16
    i16 = mybir.dt.int16
    Act = mybir.ActivationFunctionType

    nc = bacc.Bacc("TRN2", target_bir_lowering=False, debug=False, num_swdge_queues=4)

    embed_t = nc.dram_tensor("embed_t", [V, EPAD], bf16, kind="ExternalInput").ap()
    idxw = nc.dram_tensor("idxw", [128, 520], i16, kind="ExternalInput").ap()
    alpha_r = nc.dram_tensor("alpha_r", [128, NT], f32, kind="ExternalInput").ap()
    sent_f = nc.dram_tensor("sent_f", [128, NT], f32, kind="ExternalInput").ap()
    ws_pad = nc.dram_tensor("ws_pad", [EPAD, C], bf16, kind="ExternalInput").ap()
    wa_pad = nc.dram_tensor("wa_pad", [EPAD, C], bf16, kind="ExternalInput").ap()
    watt_t = nc.dram_tensor("watt_t", [D, C], bf16, kind="ExternalInput").ap()
    gw_t = nc.dram_tensor("gw_t", [D, C], bf16, kind="ExternalInput").ap()
    wst = nc.dram_tensor("wst", [C, D], bf16, kind="ExternalInput").ap()
    guide_c = nc.dram_tensor("guide_c", [K, C], f32, kind="ExternalInput").ap()
    scale_t = nc.dram_tensor("scale_t", [1, 1], f32, kind="ExternalInput").ap()
    id16 = nc.dram_tensor("id16", [16, 16], bf16, kind="ExternalInput").ap()
    out_d = nc.dram_tensor("out", [K, BPC], f32, kind="ExternalOutput").ap()

    DCH = [(0, 128), (128, 128), (256, 44)]

    with tile.TileContext(nc) as tc:
        consts = tc.alloc_tile_pool(name="consts", bufs=1)
        wpool = tc.alloc_tile_pool(name="wpool", bufs=1)

        # ---- constant loads ------------------------------------------------
        idx_sb = consts.tile([128, 520], i16)
        nc.sync.dma_start(out=idx_sb[:], in_=idxw)
        alpha_sb = consts.tile([128, NT], f32)
        nc.sync.dma_start(out=alpha_sb[:], in_=alpha_r)
        sent_sb = consts.tile([128, NT], f32)
        nc.sync.dma_start(out=sent_sb[:], in_=sent_f)
        id_sb = consts.tile([16, 16], bf16)
        nc.sync.dma_start(out=id_sb[:], in_=id16)
        sc_sb = consts.tile([1, 1], f32)
        nc.sync.dma_start(out=sc_sb[:], in_=scale_t)

        mask_sb = consts.tile([128, NT], f32)
        nc.vector.tensor_scalar_min(mask_sb[:], sent_sb[:], 1.0)
        al2_sb = consts.tile([128, NT], f32)
        nc.vector.tensor_mul(al2_sb[:], alpha_sb[:], alpha_sb[:])

        eps_col = consts.tile([128, 1], f32)
        nc.vector.memset(eps_col[:], SQUASH_EPS)
        ones_col = consts.tile([128, 1], f32)
        nc.vector.memset(ones_col[:], 1.0)

        # combo rhs tiles [128, 319] bf16: [W_s | v_0..v_15 | M]
        combo = []
        for m in range(3):
            t = wpool.tile([128, 319], bf16, tag=f"combo{m}")
            nc.sync.dma_start(out=t[:, 0:300], in_=ws_pad[128 * m : 128 * (m + 1), :])
            combo.append(t)
        wa_tiles = []
        for m in range(3):
            t = wpool.tile([128, C], bf16, tag=f"wa{m}")
            nc.sync.dma_start(out=t[:], in_=wa_pad[128 * m : 128 * (m + 1), :])
            wa_tiles.append(t)

        def load_chunks(name, src, cols):
            tiles = []
            for m, (st, sz) in enumerate(DCH):
                t = wpool.tile([sz, cols], bf16, tag=f"{name}{m}")
                nc.sync.dma_start(out=t[:], in_=src[st : st + sz, :])
                tiles.append(t)
            return tiles

        # zero the never-written pad rows of combo[2]'s G columns
        nc.vector.memset(combo[2][:, 300:319], 0.0)
        watt_tiles = load_chunks("watt", watt_t, C)  # [d, c]
        gw_tiles = load_chunks("gwt", gw_t, C)  # [d', c]
        wst_tiles = load_chunks("wst", wst, D)  # [c, d]

        sexp_all = consts.tile([128, NT], f32)
        sq_all = consts.tile([128, NT], f32)
        rawsq_all = consts.tile([K, BPC], f32)

        # big phase-1 output: raw (pre-squash) primary+scores, bf16
        prim_all = consts.tile([128, NT, 304], bf16)

        # ---- gathers (transposed): out[d%128, d//128, tok] -----------------
        gpool = tc.alloc_tile_pool(name="gpool", bufs=1)
        gat = []
        for g in range(NG):
            t = gpool.tile([128, 3, GCH], bf16, tag=f"gat{g}")
            nc.gpsimd.dma_gather(
                t[:],
                embed_t,
                idx_sb[:, (GCH // 16) * g : (GCH // 16) * (g + 1)],
                GCH,
                GCH,
                EPAD,
                transpose=True,
                queue_num=g % 4,
            )
            gat.append(t)
        gat_asp = gpool.tile([128, 3, 128], bf16, tag="gasp")
        nc.gpsimd.dma_gather(
            gat_asp[:], embed_t, idx_sb[:, 512:520], 128, BPC, EPAD, transpose=True
        )

        def factor_ops(pool, sqv, n, w, tag, extra_mul=None):
            """f = sq/(1+sq)/sqrt(sq+eps) (* extra_mul) for sq [n, w]."""
            t1 = pool.tile([n, w], f32, tag=f"{tag}t1")
            r1 = pool.tile([n, w], f32, tag=f"{tag}r1")
            s2 = pool.tile([n, w], f32, tag=f"{tag}s2")
            r2 = pool.tile([n, w], f32, tag=f"{tag}r2")
            fq = pool.tile([n, w], f32, tag=f"{tag}fq")
            nc.vector.tensor_scalar_add(t1[:], sqv, 1.0)
            nc.vector.reciprocal(r1[:], t1[:])
            nc.scalar.activation(s2[:], sqv, Act.Sqrt, bias=eps_col[:n, 0:1])
            nc.vector.reciprocal(r2[:], s2[:])
            nc.vector.tensor_mul(fq[:], r1[:], r2[:])
            nc.vector.tensor_mul(fq[:], fq[:], sqv)
            if extra_mul is not None:
                nc.vector.tensor_mul(fq[:], fq[:], extra_mul)
            return fq

        # ---- prologue: aspect capsule, guide, G ----------------------------
        with (
            tc.tile_pool(name="pro_ps", bufs=1, space="PSUM") as pps,
            tc.tile_pool(name="pro_sb", bufs=2) as psb,
        ):
            # asp_cap = squash(asp @ W_a)   psum [16, 300]
            acap_ps = pps.tile([BPC, C], f32, tag="acap_ps")
            for m in range(3):
                nc.tensor.matmul(
                    acap_ps[:],
                    gat_asp[:, m, 0:BPC],
                    wa_tiles[m][:],
                    start=(m == 0),
                    stop=(m == 2),
                )
            asq = psb.tile([BPC, 1], f32, tag="asq")
            ascr = psb.tile([BPC, C], f32, tag="ascr")
            nc.scalar.activation(
                out=ascr[:], in_=acap_ps[:], func=Act.Square, accum_out=asq[:]
            )
            af = factor_ops(psb, asq[:], BPC, 1, "asp")
            acap = psb.tile([BPC, C], bf16, tag="acap")
            nc.vector.tensor_scalar_mul(acap[:], acap_ps[:], af[:])

            # asp_capT chunks [d, 16] via matmul-with-identity transpose
            acapT = []
            for m, (st, sz) in enumerate(DCH):
                tp = pps.tile([128, 16], f32, tag="acapT_ps")
                nc.tensor.matmul(
                    tp[:sz, :],
                    acap[:, st : st + sz],
                    id_sb[:],
                    start=True,
                    stop=True,
                )
                t = psb.tile([sz, 16], bf16, tag=f"acapT{m}")
                nc.scalar.copy(out=t[:], in_=tp[:sz, :])
                acapT.append(t)

            # guide = squash(guide_capsule) -> guideT chunks [d', 3]
            graw = psb.tile([K, C], f32, tag="graw")
            nc.sync.dma_start(out=graw[:], in_=guide_c)
            gsq = psb.tile([K, 1], f32, tag="gsq")
            gscr = psb.tile([K, C], f32, tag="gscr")
            nc.scalar.activation(
                out=gscr[:], in_=graw[:], func=Act.Square, accum_out=gsq[:]
            )
            gf = factor_ops(psb, gsq[:], K, 1, "gd")
            gsb = psb.tile([K, C], bf16, tag="gsb")
            nc.vector.tensor_scalar_mul(gsb[:], graw[:], gf[:])
            guideT = []
            for m, (st, sz) in enumerate(DCH):
                tp = pps.tile([128, K], f32, tag="guideT_ps")
                nc.tensor.matmul(
                    tp[:sz, :],
                    gsb[:, st : st + sz],
                    id_sb[0:K, 0:K],
                    start=True,
                    stop=True,
                )
                t = psb.tile([sz, K], bf16, tag=f"guideT{m}")
                nc.scalar.copy(out=t[:], in_=tp[:sz, :])
                guideT.append(t)

            # stack[c, 19] = [vT | M]: vT = W_att @ asp_cap, M = gw @ guideT
            stack = []
            for m, (st, sz) in enumerate(DCH):
                vp = pps.tile([128, 16], f32, tag="v_ps")
                mp = pps.tile([128, K], f32, tag="m_ps")
                for kk, (kst, ksz) in enumerate(DCH):
                    nc.tensor.matmul(
                        vp[:sz, :],
                        watt_tiles[kk][:, st : st + sz],
                        acapT[kk][:],
                        start=(kk == 0),
                        stop=(kk == 2),
                    )
                for kk, (kst, ksz) in enumerate(DCH):
                    nc.tensor.matmul(
                        mp[:sz, :],
                        gw_tiles[kk][:, st : st + sz],
                        guideT[kk][:],
                        start=(kk == 0),
                        stop=(kk == 2),
                    )
                t = psb.tile([sz, 19], bf16, tag=f"stack{m}")
                nc.scalar.copy(out=t[:, 0:16], in_=vp[:sz, :])
                nc.scalar.copy(out=t[:, 16:19], in_=mp[:sz, :])
                stack.append(t)

            # G = W_s.T-chunks @ stack : [d, 19] per d chunk
            g_sb = []
            for m, (st, sz) in enumerate(DCH):
                gp = pps.tile([128, 19], f32, tag="g_ps")
                for kk, (kst, ksz) in enumerate(DCH):
                    nc.tensor.matmul(
                        gp[:sz, :],
                        wst_tiles[kk][:, st : st + sz],
                        stack[kk][:],
                        start=(kk == 0),
                        stop=(kk == 2),
                    )
                t = wpool.tile([sz, 19], bf16, tag=f"g_sb{m}")
                nc.scalar.copy(out=t[:], in_=gp[:sz, :])
                g_sb.append(t)

            # combo cols 300..315 = v columns, 316..318 = M-projection
            for m, (st, sz) in enumerate(DCH):
                nc.vector.tensor_copy(combo[m][:sz, 300:316], g_sb[m][:, 0:16])
                nc.vector.tensor_copy(combo[m][:sz, 316:319], g_sb[m][:, 16:19])

        # ---- phase 1: matmuls + raw psum->sbuf copies ----------------------
        with tc.tile_pool(name="pp_ps", bufs=6, space="PSUM") as pps2:
            for e in range(BPC):
                for j in range(4):
                    jj = 4 * e + j
                    g, r = jj // 4, jj % 4
                    pp = pps2.tile([128, 319], f32, tag="pp")
                    for m in range(3):
                        nc.tensor.matmul(
                            pp[:],
                            gat[g][:, m, 128 * r : 128 * (r + 1)],
                            combo[m][:],
                            start=(m == 0),
                            stop=(m == 2),
                        )
                    # split copies across DVE/ACT by tile parity
                    big = nc.vector.tensor_copy if jj % 2 == 0 else nc.scalar.copy
                    sml = nc.scalar.copy if jj % 2 == 0 else nc.vector.tensor_copy
                    if jj % 2 == 0:
                        big(prim_all[:, jj, 0:300], pp[:, 0:300])
                        sml(out=prim_all[:, jj, 300:301], in_=pp[:, 300 + e : 301 + e])
                        sml(out=prim_all[:, jj, 301:304], in_=pp[:, 316:319])
                    else:
                        big(out=prim_all[:, jj, 0:300], in_=pp[:, 0:300])
                        sml(prim_all[:, jj, 300:301], pp[:, 300 + e : 301 + e])
                        sml(prim_all[:, jj, 301:304], pp[:, 316:319])

        # ---- phase 2: batched squash/softmax + capsule ---------------------
        with (
            tc.tile_pool(name="cp_ps", bufs=2, space="PSUM") as cps,
            tc.tile_pool(name="p2sb", bufs=1) as p2sb,
        ):
            # sum of squares over C from the stored raw primary (bf16)
            sqscr = p2sb.tile([128, NT, C], bf16, tag="sqscr")
            nc.vector.tensor_mul(
                sqscr[:], prim_all[:, :, 0:300], prim_all[:, :, 0:300]
            )
            nc.vector.reduce_sum(sq_all[:], sqscr[:], axis=mybir.AxisListType.X)
            # squash factors for all tokens: sq = rawsq*alpha^2
            sqs = p2sb.tile([128, NT], f32, tag="sqs")
            nc.vector.tensor_mul(sqs[:], sq_all[:], al2_sb[:])
            fq = factor_ops(p2sb, sqs[:], 128, NT, "mn", extra_mul=alpha_sb[:])

            # scores/gm: sg = raw * f  (strided view over prim_all)
            sg = p2sb.tile([128, NT, 4], f32, tag="sg")
            nc.vector.tensor_tensor(
                out=sg[:],
                in0=prim_all[:, :, 300:304],
                in1=fq[:].to_broadcast([128, NT, 4]),
                op=mybir.AluOpType.mult,
            )
            esg = p2sb.tile([128, NT, 4], f32, tag="esg")
            nc.scalar.activation(esg[:], sg[:], Act.Exp)

            gsum = p2sb.tile([128, NT], f32, tag="gsum")
            nc.vector.reduce_sum(gsum[:], esg[:, :, 1:4], axis=mybir.AxisListType.X)
            ginv = p2sb.tile([128, NT], f32, tag="ginv")
            nc.vector.reciprocal(ginv[:], gsum[:])
            nc.vector.tensor_mul(sexp_all[:], esg[:, :, 0], mask_sb[:])
            gt = p2sb.tile([128, NT], f32, tag="gt")
            nc.vector.tensor_mul(gt[:], ginv[:], sexp_all[:])
            nc.vector.tensor_mul(gt[:], gt[:], fq[:])
            gmc = p2sb.tile([128, NT, K], bf16, tag="gmc")
            nc.vector.tensor_tensor(
                out=gmc[:],
                in0=esg[:, :, 1:4],
                in1=gt[:].to_broadcast([128, NT, K]),
                op=mybir.AluOpType.mult,
            )

            # capsule accumulation per example
            for e in range(BPC):
                cp = cps.tile([K, C], f32, tag="cp")
                for j in range(4):
                    jj = 4 * e + j
                    nc.tensor.matmul(
                        cp[:],
                        gmc[:, jj, :],
                        prim_all[:, jj, 0:300],
                        start=(j == 0),
                        stop=(j == 3),
                    )
                sc3 = p2sb.tile([K, C], f32, tag="sc3")
                nc.scalar.activation(
                    out=sc3[:],
                    in_=cp[:],
                    func=Act.Square,
                    accum_out=rawsq_all[:, e : e + 1],
                )

        # ---- epilogue -------------------------------------------------------
        with (
            tc.tile_pool(name="ep_ps", bufs=1, space="PSUM") as eps_ps,
            tc.tile_pool(name="ep_sb", bufs=1) as esb,
        ):
            ssum_ps = eps_ps.tile([1, NT], f32)
            nc.tensor.matmul(
                ssum_ps[:], ones_col[:], sexp_all[:], start=True, stop=True
            )
            ssum = esb.tile([1, NT], f32)
            nc.scalar.copy(out=ssum[:], in_=ssum_ps[:])
            esum = esb.tile([1, BPC], f32)
            nc.vector.reduce_sum(
                esum[:],
                ssum[:].rearrange("p (a b) -> p a b", b=4),
                axis=mybir.AxisListType.X,
            )
            inv = esb.tile([1, BPC], f32)
            nc.vector.reciprocal(inv[:], esum[:])
            inv_sc = esb.tile([1, BPC + 1], f32)
            nc.vector.tensor_mul(inv_sc[:, 0:BPC], inv[:], inv[:])
            nc.vector.tensor_mul(inv_sc[:, BPC : BPC + 1], sc_sb[:], sc_sb[:])
            ones3 = esb.tile([1, K], f32)
            nc.vector.memset(ones3[:], 1.0)
            bc_ps = eps_ps.tile([K, BPC + 1], f32)
            nc.tensor.matmul(bc_ps[:], ones3[:], inv_sc[:], start=True, stop=True)
            bc_sb = esb.tile([K, BPC + 1], f32)
            nc.scalar.copy(out=bc_sb[:], in_=bc_ps[:])

            vv = esb.tile([K, BPC], f32)
            nc.vector.tensor_mul(vv[:], rawsq_all[:], bc_sb[:, 0:BPC])
            nc.vector.tensor_scalar_mul(vv[:], vv[:], bc_sb[:, BPC : BPC + 1])

            a1 = esb.tile([K, BPC], f32)
            ra = esb.tile([K, BPC], f32)
            sv = esb.tile([K, BPC], f32)
            sve = esb.tile([K, BPC], f32)
            rsve = esb.tile([K, BPC], f32)
            t1 = esb.tile([K, BPC], f32)
            t2 = esb.tile([K, BPC], f32)
            outsb = esb.tile([K, BPC], f32)
            nc.vector.tensor_scalar_add(a1[:], vv[:], 1.0)
            nc.vector.reciprocal(ra[:], a1[:])
            nc.scalar.activation(sv[:], vv[:], Act.Sqrt)
            nc.scalar.activation(sve[:], vv[:], Act.Sqrt, bias=eps_col[:K, 0:1])
            nc.vector.reciprocal(rsve[:], sve[:])
            nc.vector.tensor_mul(t1[:], vv[:], ra[:])
            nc.vector.tensor_mul(t2[:], sv[:], rsve[:])
            nc.vector.tensor_mul(outsb[:], t1[:], t2[:])
            nc.sync.dma_start(out=out_d, in_=outsb[:])

        gpool.release()
        wpool.release()
        consts.release()

    nc.compile()
    return nc


def _marshal(inputs):
    """Host-side sharding/layout marshalling. Returns per-core in_maps."""
    import ml_dtypes

    bf = ml_dtypes.bfloat16

    sentence = np.asarray(inputs["sentence"])
    aspect = np.asarray(inputs["aspect"])
    alpha = np.asarray(inputs["category_alpha"], dtype=np.float32)
    embed = np.asarray(inputs["embed"], dtype=np.float32)
    W_s = np.asarray(inputs["W_s"], dtype=np.float32)
    b_s = np.asarray(inputs["b_s"], dtype=np.float32)
    assert np.all(b_s == 0), "kernel drops b_s (always zero in this problem)"
    W_a = np.asarray(inputs["W_a"], dtype=np.float32)
    b_a = np.asarray(inputs["b_a"], dtype=np.float32)
    assert np.all(b_a == 0), "kernel drops b_a (always zero in this problem)"
    W_att = np.asarray(inputs["W_att"], dtype=np.float32)
    guide_capsule = np.asarray(inputs["guide_capsule"], dtype=np.float32)
    guide_weight = np.asarray(inputs["guide_weight"], dtype=np.float32)
    scale = np.asarray(inputs["scale"], dtype=np.float32).reshape(1, 1)

    embed_t = np.zeros((V, EPAD), bf)
    embed_t[:, :D] = embed.astype(bf)
    ws_pad = np.zeros((EPAD, C), bf)
    ws_pad[:D] = W_s.astype(bf)
    wa_pad = np.zeros((EPAD, C), bf)
    wa_pad[:D] = W_a.astype(bf)

    shared = {
        "embed_t": embed_t,
        "ws_pad": ws_pad,
        "wa_pad": wa_pad,
        "watt_t": np.ascontiguousarray(W_att.T).astype(bf),
        "gw_t": np.ascontiguousarray(guide_weight.T).astype(bf),
        "wst": np.ascontiguousarray(W_s.T).astype(bf),
        "guide_c": np.ascontiguousarray(guide_capsule),
        "scale_t": scale,
        "id16": np.eye(16, dtype=np.float32).astype(bf),
    }

    in_maps = []
    for c in range(NCORES):
        sent_c = sentence[c * BPC : (c + 1) * BPC].reshape(-1)  # [8192]
        asp_c = aspect[c * BPC : (c + 1) * BPC].reshape(-1)  # [16]
        tok = np.full(8320, -1, np.int64)
        tok[:TPC] = sent_c
        tok[TPC : TPC + BPC] = asp_c
        idxw_np = np.tile(tok.reshape(520, 16).T.astype(np.int16), (8, 1))
        alpha_c = alpha[c * BPC : (c + 1) * BPC].reshape(-1)
        alpha_rr = np.ascontiguousarray(alpha_c.reshape(NT, 128).T)
        sent_rr = np.ascontiguousarray(sent_c.reshape(NT, 128).T.astype(np.float32))
        m = dict(shared)
        m["idxw"] = idxw_np
        m["alpha_r"] = alpha_rr
        m["sent_f"] = sent_rr
        in_maps.append(m)
    return in_maps


def kernel(**inputs) -> np.ndarray:
    from concourse.bass_utils import run_bass_kernel_spmd

    if "nc" not in _CACHE:
        _CACHE["nc"] = _build()
    nc = _CACHE["nc"]
    in_maps = _marshal(inputs)
    res = run_bass_kernel_spmd(nc, in_maps, list(range(NCORES)))
    out = np.zeros((B, K), np.float32)
    for c in range(NCORES):
        oc = res.results[c]["out"]  # [K, BPC]
        out[c * BPC : (c + 1) * BPC, :] = oc.T
    return out


# revision 22
# speedup vs baseline: 1.2264x; 1.2264x over previous
"""Trainium2 Bass kernel for nn_CapsuleNetwork (capsule-guided aspect routing).

Data-parallel over batch: B=128 examples sharded 16-per-core across 8 cores.
Self-contained: hardcodes all shapes; host side only shards/marshals inputs.

Math (per example, equal to reference.py up to fp associativity):
  mask = sentence != 0
  caml = embed[sentence] * alpha                      [S, D]
  primary = squash(caml @ W_s)                        [S, C]   squash over C
  asp_cap = squash(embed[aspect] @ W_a)               [C]
  v = W_att @ asp_cap                                 [C]
  scores = primary @ v ; sexp = exp(scores)*mask
  M = guide_weight @ squash(guide_capsule).T          [C, K]
  gm = softmax_k(primary @ M) * sexp[:, None]
  cc_raw[k, :] = sum_s gm[s, k] * primary[s, :]       [K, C]
  out[k] = g(scale^2 * (1/sum(sexp))^2 * |cc_raw[k]|^2)
  with g(q) = q/(1+q) * sqrt(q)/sqrt(q+eps)
(b_s / b_a are asserted zero — the task generator hardcodes them to zero.)

Implementation notes:
 * embed table is padded to 384 cols, cast to bf16 and gathered with
   dma_gather(transpose=True): the gather output IS the transposed
   [d-chunk, token] layout the PE needs — no on-chip transposes at all.
 * alpha and the squash factor are folded into per-token scalars applied
   AFTER the matmul (squash(alpha*x @ W) = (alpha-aware factor) * (x @ W)).
 * one [128,304] matmul group per token tile computes primary-linear plus
   the scores/gm projections (extra 4 columns = W_s @ [v_e | M]).
 * phase 1 (per tile): 3 matmuls, ACT square+accum, raw psum->sbuf copy.
   phase 2 (batched at [128,64]): squash factors, exp, softmax, capsule
   matmuls.  This keeps ACT on one table per phase and slashes the count
   of small vector ops.
"""

import numpy as np

B, S, D, C, K, V = 128, 512, 300, 300, 3, 30000
NCORES = 8
BPC = B // NCORES  # examples per core = 16
TPC = BPC * S  # tokens per core = 8192
NT = TPC // 128  # 64 token tiles per core
EPAD = 384  # bf16 row: 384*2 = 768B (256B-aligned)
GCH = 512  # tokens per gather call (transpose mode crashes above ~768)
NG = TPC // GCH  # 8 gather calls
SQUASH_EPS = 1e-8

_CACHE = {}


def _build():
    import concourse.bacc as bacc
    import concourse.mybir as mybir
    import concourse.tile as tile

    f32 = mybir.dt.float32
    bf16 = mybir.dt.bfloat16
    i16 = mybir.dt.int16
    Act = mybir.ActivationFunctionType

    nc = bacc.Bacc("TRN2", target_bir_lowering=False, debug=False, num_swdge_queues=4)

    embed_t = nc.dram_tensor("embed_t", [V, EPAD], bf16, kind="ExternalInput").ap()
    idxw = nc.dram_tensor("idxw", [128, 520], i16, kind="ExternalInput").ap()
    alpha_r = nc.dram_tensor("alpha_r", [128, NT], f32, kind="ExternalInput").ap()
    sent_f = nc.dram_tensor("sent_f", [128, NT], f32, kind="ExternalInput").ap()
    ws_pad = nc.dram_tensor("ws_pad", [EPAD, C], bf16, kind="ExternalInput").ap()
    wa_pad = nc.dram_tensor("wa_pad", [EPAD, C], bf16, kind="ExternalInput").ap()
    watt_t = nc.dram_tensor("watt_t", [D, C], bf16, kind="ExternalInput").ap()
    gw_t = nc.dram_tensor("gw_t", [D, C], bf16, kind="ExternalInput").ap()
    wst = nc.dram_tensor("wst", [C, D], bf16, kind="ExternalInput").ap()
    guide_c = nc.dram_tensor("guide_c", [K, C], f32, kind="ExternalInput").ap()
    scale_t = nc.dram_tensor("scale_t", [1, 1], f32, kind="ExternalInput").ap()
    id16 = nc.dram_tensor("id16", [16, 16], bf16, kind="ExternalInput").ap()
    out_d = nc.dram_tensor("out", [K, BPC], f32, kind="ExternalOutput").ap()

    DCH = [(0, 128), (128, 128), (256, 44)]

    with tile.TileContext(nc) as tc:
        consts = tc.alloc_tile_pool(name="consts", bufs=1)
        wpool = tc.alloc_tile_pool(name="wpool", bufs=1)

        # ---- constant loads ------------------------------------------------
        idx_sb = consts.tile([128, 520], i16)
        nc.sync.dma_start(out=idx_sb[:], in_=idxw)
        alpha_sb = consts.tile([128, NT], f32)
        nc.sync.dma_start(out=alpha_sb[:], in_=alpha_r)
        sent_sb = consts.tile([128, NT], f32)
        nc.sync.dma_start(out=sent_sb[:], in_=sent_f)
        id_sb = consts.tile([16, 16], bf16)
        nc.sync.dma_start(out=id_sb[:], in_=id16)
        sc_sb = consts.tile([1, 1], f32)
        nc.sync.dma_start(out=sc_sb[:], in_=scale_t)

        mask_sb = consts.tile([128, NT], f32)
        nc.vector.tensor_scalar_min(mask_sb[:], sent_sb[:], 1.0)
        al2_sb = consts.tile([128, NT], f32)
        nc.vector.tensor_mul(al2_sb[:], alpha_sb[:], alpha_sb[:])

        eps_col = consts.tile([128, 1], f32)
        nc.vector.memset(eps_col[:], SQUASH_EPS)
        ones_col = consts.tile([128, 1], f32)
        nc.vector.memset(ones_col[:], 1.0)

        # combo rhs tiles [128, 319] bf16: [W_s | v_0..v_15 | M]
        combo = []
        for m in range(3):
            t = wpool.tile([128, 319], bf16, tag=f"combo{m}")
            nc.sync.dma_start(out=t[:, 0:300], in_=ws_pad[128 * m : 128 * (m + 1), :])
            combo.append(t)
        wa_tiles = []
        for m in range(3):
            t = wpool.tile([128, C], bf16, tag=f"wa{m}")
            nc.sync.dma_start(out=t[:], in_=wa_pad[128 * m : 128 * (m + 1), :])
            wa_tiles.append(t)

        def load_chunks(name, src, cols):
            tiles = []
            for m, (st, sz) in enumerate(DCH):
                t = wpool.tile([sz, cols], bf16, tag=f"{name}{m}")
                nc.sync.dma_start(out=t[:], in_=src[st : st + sz, :])
                tiles.append(t)
            return tiles

        # zero the never-written pad rows of combo[2]'s G columns
        nc.vector.memset(combo[2][:, 300:319], 0.0)
        watt_tiles = load_chunks("watt", watt_t, C)  # [d, c]
        gw_tiles = load_chunks("gwt", gw_t, C)  # [d', c]
        wst_tiles = load_chunks("wst", wst, D)  # [c, d]

        sexp_all = consts.tile([128, NT], f32)
        sq_all = consts.tile([128, NT], f32)
        rawsq_all = consts.tile([K, BPC], f32)

        # big phase-1 output: raw (pre-squash) primary+scores, bf16
        prim_all = consts.tile([128, NT, 319], bf16)

        # ---- gathers (transposed): out[d%128, d//128, tok] -----------------
        gpool = tc.alloc_tile_pool(name="gpool", bufs=1)
        gat_asp = gpool.tile([128, 3, 128], bf16, tag="gasp")
        nc.gpsimd.dma_gather(
            gat_asp[:], embed_t, idx_sb[:, 512:520], 128, BPC, EPAD, transpose=True
        )
        gat = []
        for g in range(NG):
            t = gpool.tile([128, 3, GCH], bf16, tag=f"gat{g}")
            nc.gpsimd.dma_gather(
                t[:],
                embed_t,
                idx_sb[:, (GCH // 16) * g : (GCH // 16) * (g + 1)],
                GCH,
                GCH,
                EPAD,
                transpose=True,
                queue_num=(g + 1) % 4,
            )
            gat.append(t)

        def factor_ops(pool, sqv, n, w, tag, extra_mul=None):
            """f = sq/(1+sq)/sqrt(sq+eps) (* extra_mul) for sq [n, w]."""
            t1 = pool.tile([n, w], f32, tag=f"{tag}t1")
            r1 = pool.tile([n, w], f32, tag=f"{tag}r1")
            s2 = pool.tile([n, w], f32, tag=f"{tag}s2")
            r2 = pool.tile([n, w], f32, tag=f"{tag}r2")
            fq = pool.tile([n, w], f32, tag=f"{tag}fq")
            nc.vector.tensor_scalar_add(t1[:], sqv, 1.0)
            nc.vector.reciprocal(r1[:], t1[:])
            nc.scalar.activation(s2[:], sqv, Act.Sqrt, bias=eps_col[:n, 0:1])
            nc.vector.reciprocal(r2[:], s2[:])
            nc.vector.tensor_mul(fq[:], r1[:], r2[:])
            nc.vector.tensor_mul(fq[:], fq[:], sqv)
            if extra_mul is not None:
                nc.vector.tensor_mul(fq[:], fq[:], extra_mul)
            return fq

        # ---- prologue: aspect capsule, guide, G ----------------------------
        with (
            tc.tile_pool(name="pro_ps", bufs=1, space="PSUM") as pps,
            tc.tile_pool(name="pro_sb", bufs=2) as psb,
        ):
            # asp_cap = squash(asp @ W_a)   psum [16, 300]
            acap_ps = pps.tile([BPC, C], f32, tag="acap_ps")
            for m in range(3):
                nc.tensor.matmul(
                    acap_ps[:],
                    gat_asp[:, m, 0:BPC],
                    wa_tiles[m][:],
                    start=(m == 0),
                    stop=(m == 2),
                )
            asq = psb.tile([BPC, 1], f32, tag="asq")
            ascr = psb.tile([BPC, C], f32, tag="ascr")
            nc.scalar.activation(
                out=ascr[:], in_=acap_ps[:], func=Act.Square, accum_out=asq[:]
            )
            af = factor_ops(psb, asq[:], BPC, 1, "asp")
            acap = psb.tile([BPC, C], bf16, tag="acap")
            nc.vector.tensor_scalar_mul(acap[:], acap_ps[:], af[:])

            # asp_capT chunks [d, 16] via matmul-with-identity transpose
            acapT = []
            for m, (st, sz) in enumerate(DCH):
                tp = pps.tile([128, 16], f32, tag="acapT_ps")
                nc.tensor.matmul(
                    tp[:sz, :],
                    acap[:, st : st + sz],
                    id_sb[:],
                    start=True,
                    stop=True,
                )
                t = psb.tile([sz, 16], bf16, tag=f"acapT{m}")
                nc.scalar.copy(out=t[:], in_=tp[:sz, :])
                acapT.append(t)

            # guide = squash(guide_capsule) -> guideT chunks [d', 3]
            graw = psb.tile([K, C], f32, tag="graw")
            nc.sync.dma_start(out=graw[:], in_=guide_c)
            gsq = psb.tile([K, 1], f32, tag="gsq")
            gscr = psb.tile([K, C], f32, tag="gscr")
            nc.scalar.activation(
                out=gscr[:], in_=graw[:], func=Act.Square, accum_out=gsq[:]
            )
            gf = factor_ops(psb, gsq[:], K, 1, "gd")
            gsb = psb.tile([K, C], bf16, tag="gsb")
            nc.vector.tensor_scalar_mul(gsb[:], graw[:], gf[:])
            guideT = []
            for m, (st, sz) in enumerate(DCH):
                tp = pps.tile([128, K], f32, tag="guideT_ps")
                nc.tensor.matmul(
                    tp[:sz, :],
                    gsb[:, st : st + sz],
                    id_sb[0:K, 0:K],
                    start=True,
                    stop=True,
                )
                t = psb.tile([sz, K], bf16, tag=f"guideT{m}")
                nc.scalar.copy(out=t[:], in_=tp[:sz, :])
                guideT.append(t)

            # stack[c, 19] = [vT | M]: vT = W_att @ asp_cap, M = gw @ guideT
            stack = []
            for m, (st, sz) in enumerate(DCH):
                vp = pps.tile([128, 16], f32, tag="v_ps")
                mp = pps.tile([128, K], f32, tag="m_ps")
                for kk, (kst, ksz) in enumerate(DCH):
                    nc.tensor.matmul(
                        vp[:sz, :],
                        watt_tiles[kk][:, st : st + sz],
                        acapT[kk][:],
                        start=(kk == 0),
                        stop=(kk == 2),
                    )
                for kk, (kst, ksz) in enumerate(DCH):
                    nc.tensor.matmul(
                        mp[:sz, :],
                        gw_tiles[kk][:, st : st + sz],
                        guideT[kk][:],
                        start=(kk == 0),
                        stop=(kk == 2),
                    )
                t = psb.tile([sz, 19], bf16, tag=f"stack{m}")
                nc.scalar.copy(out=t[:, 0:16], in_=vp[:sz, :])
                nc.scalar.copy(out=t[:, 16:19], in_=mp[:sz, :])
                stack.append(t)

            # G = W_s.T-chunks @ stack : [d, 19] per d chunk
            g_sb = []
            for m, (st, sz) in enumerate(DCH):
                gp = pps.tile([128, 19], f32, tag="g_ps")
                for kk, (kst, ksz) in enumerate(DCH):
                    nc.tensor.matmul(
                        gp[:sz, :],
                        wst_tiles[kk][:, st : st + sz],
                        stack[kk][:],
                        start=(kk == 0),
                        stop=(kk == 2),
                    )
                t = wpool.tile([sz, 19], bf16, tag=f"g_sb{m}")
                nc.scalar.copy(out=t[:], in_=gp[:sz, :])
                g_sb.append(t)

            # combo cols 300..315 = v columns, 316..318 = M-projection
            for m, (st, sz) in enumerate(DCH):
                nc.vector.tensor_copy(combo[m][:sz, 300:316], g_sb[m][:, 0:16])
                nc.vector.tensor_copy(combo[m][:sz, 316:319], g_sb[m][:, 16:19])

        # ---- phase 1: matmuls + raw psum->sbuf copies ----------------------
        with (
            tc.tile_pool(name="pp_ps", bufs=6, space="PSUM") as pps2,
            tc.tile_pool(name="p1sb", bufs=3) as p1sb,
        ):
            for e in range(BPC):
                for j in range(4):
                    jj = 4 * e + j
                    g, r = jj // 4, jj % 4
                    pp = pps2.tile([128, 319], f32, tag="pp")
                    for m in range(3):
                        nc.tensor.matmul(
                            pp[:],
                            gat[g][:, m, 128 * r : 128 * (r + 1)],
                            combo[m][:],
                            start=(m == 0),
                            stop=(m == 2),
                        )
                    sqscr = p1sb.tile([128, C], bf16, tag="sqscr")
                    nc.scalar.activation(sqscr[:], pp[:, 0:300], Act.Square)
                    nc.vector.reduce_sum(
                        sq_all[:, jj : jj + 1],
                        sqscr[:],
                        axis=mybir.AxisListType.X,
                    )
                    if jj % 2 == 0:
                        nc.vector.tensor_copy(prim_all[:, jj, :], pp[:])
                    else:
                        nc.scalar.copy(out=prim_all[:, jj, :], in_=pp[:])

        # ---- phase 2: batched squash/softmax + capsule ---------------------
        with (
            tc.tile_pool(name="cp_ps", bufs=2, space="PSUM") as cps,
            tc.tile_pool(name="p2sb", bufs=1) as p2sb,
        ):
            # squash factors for all tokens: sq = rawsq*alpha^2
            sqs = p2sb.tile([128, NT], f32, tag="sqs")
            nc.vector.tensor_mul(sqs[:], sq_all[:], al2_sb[:])
            fq = factor_ops(p2sb, sqs[:], 128, NT, "mn", extra_mul=alpha_sb[:])

            # scores/gm: sg = raw * f  (gather per-example views of prim_all)
            sg = p2sb.tile([128, NT, 4], f32, tag="sg")
            pa4 = prim_all[:].rearrange("p (e j) c -> p e j c", e=BPC)
            sg4 = sg[:].rearrange("p (e j) c -> p e j c", e=BPC)
            fq4 = fq[:].rearrange("p (e j) -> p e j", e=BPC)
            for e in range(BPC):
                nc.vector.tensor_tensor(
                    out=sg4[:, e, :, 0:1],
                    in0=pa4[:, e, :, 300 + e : 301 + e],
                    in1=fq4[:, e, :].to_broadcast([128, 4, 1]),
                    op=mybir.AluOpType.mult,
                )
                nc.vector.tensor_tensor(
                    out=sg4[:, e, :, 1:4],
                    in0=pa4[:, e, :, 316:319],
                    in1=fq4[:, e, :].to_broadcast([128, 4, 3]),
                    op=mybir.AluOpType.mult,
                )
            esg = p2sb.tile([128, NT, 4], f32, tag="esg")
            nc.scalar.activation(esg[:], sg[:], Act.Exp)

            gsum = p2sb.tile([128, NT], f32, tag="gsum")
            nc.vector.reduce_sum(gsum[:], esg[:, :, 1:4], axis=mybir.AxisListType.X)
            ginv = p2sb.tile([128, NT], f32, tag="ginv")
            nc.vector.reciprocal(ginv[:], gsum[:])
            nc.vector.tensor_mul(sexp_all[:], esg[:, :, 0], mask_sb[:])
            gt = p2sb.tile([128, NT], f32, tag="gt")
            nc.vector.tensor_mul(gt[:], ginv[:], sexp_all[:])
            nc.vector.tensor_mul(gt[:], gt[:], fq[:])
            gmc = p2sb.tile([128, NT, K], bf16, tag="gmc")
            nc.vector.tensor_tensor(
                out=gmc[:],
                in0=esg[:, :, 1:4],
                in1=gt[:].to_broadcast([128, NT, K]),
                op=mybir.AluOpType.mult,
            )

            # capsule accumulation per example
            for e in range(BPC):
                cp = cps.tile([K, C], f32, tag="cp")
                for j in range(4):
                    jj = 4 * e + j
                    nc.tensor.matmul(
                        cp[:],
                        gmc[:, jj, :],
                        prim_all[:, jj, 0:300],
                        start=(j == 0),
                        stop=(j == 3),
                    )
                sc3 = p2sb.tile([K, C], f32, tag="sc3")
                nc.scalar.activation(
                    out=sc3[:],
                    in_=cp[:],
                    func=Act.Square,
                    accum_out=rawsq_all[:, e : e + 1],
                )

        # ---- epilogue -------------------------------------------------------
        with (
            tc.tile_pool(name="ep_ps", bufs=1, space="PSUM") as eps_ps,
            tc.tile_pool(name="ep_sb", bufs=1) as esb,
        ):
            ssum_ps = eps_ps.tile([1, NT], f32)
            nc.tensor.matmul(
                ssum_ps[:], ones_col[:], sexp_all[:], start=True, stop=True
            )
            ssum = esb.tile([1, NT], f32)
            nc.scalar.copy(out=ssum[:], in_=ssum_ps[:])
            esum = esb.tile([1, BPC], f32)
            nc.vector.reduce_sum(
                esum[:],
                ssum[:].rearrange("p (a b) -> p a b", b=4),
                axis=mybir.AxisListType.X,
            )
            inv = esb.tile([1, BPC], f32)
            nc.vector.reciprocal(inv[:], esum[:])
            inv_sc = esb.tile([1, BPC + 1], f32)
            nc.vector.tensor_mul(inv_sc[:, 0:BPC], inv[:], inv[:])
            nc.vector.tensor_mul(inv_sc[:, BPC : BPC + 1], sc_sb[:], sc_sb[:])
            ones3 = esb.tile([1, K], f32)
            nc.vector.memset(ones3[:], 1.0)
            bc_ps = eps_ps.tile([K, BPC + 1], f32)
            nc.tensor.matmul(bc_ps[:], ones3[:], inv_sc[:], start=True, stop=True)
            bc_sb = esb.tile([K, BPC + 1], f32)
            nc.scalar.copy(out=bc_sb[:], in_=bc_ps[:])

            vv = esb.tile([K, BPC], f32)
            nc.vector.tensor_mul(vv[:], rawsq_all[:], bc_sb[:, 0:BPC])
            nc.vector.tensor_scalar_mul(vv[:], vv[:], bc_sb[:, BPC : BPC + 1])

            a1 = esb.tile([K, BPC], f32)
            ra = esb.tile([K, BPC], f32)
            sv = esb.tile([K, BPC], f32)
            sve = esb.tile([K, BPC], f32)
            rsve = esb.tile([K, BPC], f32)
            t1 = esb.tile([K, BPC], f32)
            t2 = esb.tile([K, BPC], f32)
            outsb = esb.tile([K, BPC], f32)
            nc.vector.tensor_scalar_add(a1[:], vv[:], 1.0)
            nc.vector.reciprocal(ra[:], a1[:])
            nc.scalar.activation(sv[:], vv[:], Act.Sqrt)
            nc.scalar.activation(sve[:], vv[:], Act.Sqrt, bias=eps_col[:K, 0:1])
            nc.vector.reciprocal(rsve[:], sve[:])
            nc.vector.tensor_mul(t1[:], vv[:], ra[:])
            nc.vector.tensor_mul(t2[:], sv[:], rsve[:])
            nc.vector.tensor_mul(outsb[:], t1[:], t2[:])
            nc.sync.dma_start(out=out_d, in_=outsb[:])

        gpool.release()
        wpool.release()
        consts.release()

    nc.compile()
    return nc


def _marshal(inputs):
    """Host-side sharding/layout marshalling. Returns per-core in_maps."""
    import ml_dtypes

    bf = ml_dtypes.bfloat16

    sentence = np.asarray(inputs["sentence"])
    aspect = np.asarray(inputs["aspect"])
    alpha = np.asarray(inputs["category_alpha"], dtype=np.float32)
    embed = np.asarray(inputs["embed"], dtype=np.float32)
    W_s = np.asarray(inputs["W_s"], dtype=np.float32)
    b_s = np.asarray(inputs["b_s"], dtype=np.float32)
    assert np.all(b_s == 0), "kernel drops b_s (always zero in this problem)"
    W_a = np.asarray(inputs["W_a"], dtype=np.float32)
    b_a = np.asarray(inputs["b_a"], dtype=np.float32)
    assert np.all(b_a == 0), "kernel drops b_a (always zero in this problem)"
    W_att = np.asarray(inputs["W_att"], dtype=np.float32)
    guide_capsule = np.asarray(inputs["guide_capsule"], dtype=np.float32)
    guide_weight = np.asarray(inputs["guide_weight"], dtype=np.float32)
    scale = np.asarray(inputs["scale"], dtype=np.float32).reshape(1, 1)

    embed_t = np.zeros((V, EPAD), bf)
    embed_t[:, :D] = embed.astype(bf)
    ws_pad = np.zeros((EPAD, C), bf)
    ws_pad[:D] = W_s.astype(bf)
    wa_pad = np.zeros((EPAD, C), bf)
    wa_pad[:D] = W_a.astype(bf)

    shared = {
        "embed_t": embed_t,
        "ws_pad": ws_pad,
        "wa_pad": wa_pad,
        "watt_t": np.ascontiguousarray(W_att.T).astype(bf),
        "gw_t": np.ascontiguousarray(guide_weight.T).astype(bf),
        "wst": np.ascontiguousarray(W_s.T).astype(bf),
        "guide_c": np.ascontiguousarray(guide_capsule),
        "scale_t": scale,
        "id16": np.eye(16, dtype=np.float32).astype(bf),
    }

    in_maps = []
    for c in range(NCORES):
        sent_c = sentence[c * BPC : (c + 1) * BPC].reshape(-1)  # [8192]
        asp_c = aspect[c * BPC : (c + 1) * BPC].reshape(-1)  # [16]
        tok = np.full(8320, -1, np.int64)
        tok[:TPC] = sent_c
        tok[TPC : TPC + BPC] = asp_c
        idxw_np = np.tile(tok.reshape(520, 16).T.astype(np.int16), (8, 1))
        alpha_c = alpha[c * BPC : (c + 1) * BPC].reshape(-1)
        alpha_rr = np.ascontiguousarray(alpha_c.reshape(NT, 128).T)
        sent_rr = np.ascontiguousarray(sent_c.reshape(NT, 128).T.astype(np.float32))
        m = dict(shared)
        m["idxw"] = idxw_np
        m["alpha_r"] = alpha_rr
        m["sent_f"] = sent_rr
        in_maps.append(m)
    return in_maps


def kernel(**inputs) -> np.ndarray:
    from concourse.bass_utils import run_bass_kernel_spmd

    if "nc" not in _CACHE:
        _CACHE["nc"] = _build()
    nc = _CACHE["nc"]
    in_maps = _marshal(inputs)
    res = run_bass_kernel_spmd(nc, in_maps, list(range(NCORES)))
    out = np.zeros((B, K), np.float32)
    for c in range(NCORES):
        oc = res.results[c]["out"]  # [K, BPC]
        out[c * BPC : (c + 1) * BPC, :] = oc.T
    return out
